# revision 71
# baseline (speedup 1.0000x reference)
# Trainium2 Bass kernel for nn_AdaptiveCrossHadamard (v2).
#
# Reference computation (per sample):
#   y   = BN(Conv1x1(x))                                  [256, 64*64]
#   p   = mean_pixels(y); logits = conv1d(p, eca_w, k=5)  [256]
#   idx = top_32(logits) (sorted desc, ties -> lower idx)
#   xs  = y[idx]                                          [32, 4096]
#   z   = BN_s(xs[hi] * xs[hj])  for all i<j pairs        [496, 4096]
#   out = concat([y, z], channel axis)                    [752, 4096]
#
# v2 strategy (8 NeuronCores, batch-parallel, 2 samples/core):
#   - fp16 output DMA (halves HBM store traffic); host upcasts to f32.
#   - x_sel kept in a STACKED layout [128, 1024]: partition group g
#     (32 rows) holds the 32 selected channels for pixel block
#     g*1024..g*1024+1023.  This enables:
#       * xsel matmuls col-tiled 4x (tile_position=(0,32g)) - 4 concurrent
#         M=32 matmuls filling one PSUM bank across partition groups.
#       * pair matmuls row-tiled 4x (tile_position=(32g,0)) - 4 concurrent
#         K=32 matmuls (vs 1/4-utilized full-array matmuls in v1).
#   - pair Hadamard via the squares trick with a K=32 NEGATED Q matrix
#     (shift folded into the DVE combine op's per-partition scalar):
#       psS = sqrt(ss/2)(xi+xj); sq = Square(psS)  [ACT, one pass]
#       psQ = -(ss/2)(xi^2+xj^2); z = (psQ + sh) + sq  [DVE stt, one pass]
#   - PSUM slots are [128,1024] f32 (2 banks), one unified 4-slot pool =
#     all 8 banks; evacuation ops run at FD=1024 to amortize fixed cost.
#   - y bias+evac split between ACT and DVE to balance engine load.
#   - DMA queues: x loads on sync HWDGE, consts on scalar HWDGE, outputs
#     on gpsimd SWDGE - three independent rings, no FIFO blocking.
import os
import sys
import numpy as np

_TRN_REPO = "/opt/trn_rl_repo"
if _TRN_REPO not in sys.path and os.path.isdir(_TRN_REPO):
    sys.path.insert(0, _TRN_REPO)

import concourse.bacc as bacc
import concourse.bass as bass
import concourse.mybir as mybir
import concourse.tile as tile
from concourse.bass_utils import run_bass_kernel_spmd

F32 = mybir.dt.float32
F16 = mybir.dt.float16
AF = mybir.ActivationFunctionType
ALU = mybir.AluOpType

B, C1, H, W = 16, 256, 64, 64
PIX = H * W                      # 4096
CS = 32
CSE = CS * (CS - 1) // 2         # 496
NCORES = 8
SPC = B // NCORES                # samples per core = 2
COUT = C1 + CSE                  # 752
EPS = 1e-5
MT4 = (CSE + 127) // 128         # 4 pair-row tiles (128,128,128,112)

# f32 const blob column layout: [128, NSEL], sections in consumer order
_WY32 = 0                        # wyT32s, 2 x 256
_BCOL = 512                      # bcol, 2 x 1
_SHC = 514                       # shift_s per m-tile, 4 x 1
_CMAT = 518                      # cmat, 2 x 256
NSEL = 1030
# fp16 blob: masks (exact in fp16) first, then weights
_TRIL = 0                        # tril, 2 x 256
_OFFD = 512                      # 1 - eye, 2 x 256
_IOTA = 1024                     # iota128 (j%32 pattern), 128
_WFOLD = 1152                    # wfold16, 2 x 256
_BC16 = 1664                     # bcol16, 2 x 1
_WY16 = 1666                     # wyT16, 2 x 256
N16 = 2178
# fp16 pair blob: [128, 992] = pS_stk (496) + pQn_stk (496)


def _build(nc: bass.Bass, dbg: bool = False):
    """Emit the per-core Tile program. SPMD: all 8 cores run this graph."""
    x_d = nc.dram_tensor("x16v", [SPC * C1, PIX], F16, kind="ExternalInput")
    xsum_d = nc.dram_tensor("xsumv", [128, 2 * SPC], F32, kind="ExternalInput")
    out_d = nc.dram_tensor("out16", [SPC * COUT, PIX], F16, kind="ExternalOutput")
    sel_d = nc.dram_tensor("selblob", [128, NSEL], F32, kind="ExternalInput")
    w16_d = nc.dram_tensor("w16blob", [128, N16], F16, kind="ExternalInput")
    pair_d = nc.dram_tensor("pairblob", [128, 2 * CSE], F16, kind="ExternalInput")
    if dbg:
        dbg_pooled = nc.dram_tensor("dbg_pooled", [SPC * C1, 1], F32,
                                    kind="ExternalOutput")
        dbg_lrow = nc.dram_tensor("dbg_lrow", [SPC, C1], F32,
                                  kind="ExternalOutput")
        dbg_rank = nc.dram_tensor("dbg_rank", [SPC * C1, 1], F32,
                                  kind="ExternalOutput")
        dbg_xsel = nc.dram_tensor("dbg_xsel", [SPC * 128, 1024], F16,
                                  kind="ExternalOutput")

    from contextlib import ExitStack
    with tile.TileContext(nc) as tc, ExitStack() as ctx:
        cpool = ctx.enter_context(tc.tile_pool(name="consts", bufs=1))
        x16p = ctx.enter_context(tc.tile_pool(name="x16", bufs=4))
        xselp = ctx.enter_context(tc.tile_pool(name="xsel", bufs=2))
        ysbp = ctx.enter_context(tc.tile_pool(name="ysb", bufs=2))
        zop = ctx.enter_context(tc.tile_pool(name="zout", bufs=5))
        sqp = ctx.enter_context(tc.tile_pool(name="sq16", bufs=6))
        gp = ctx.enter_context(tc.tile_pool(name="gwork", bufs=2))
        smallp = ctx.enter_context(tc.tile_pool(name="small", bufs=4))
        # PSUM: a 3-slot [128,1024] ring for the pair units (4-way matmul
        # waves across two 2-bank slots, FD=1024 evacuation ops) plus a
        # 2-slot [128,512] ring for y units / small sel matmuls.  The two
        # rings must stay separate: sharing one ring leashes the PE to the
        # evac engines, HAM re-throttles, and matmul time doubles.
        psp = ctx.enter_context(tc.tile_pool(name="ps", bufs=3, space="PSUM"))
        psy = ctx.enter_context(tc.tile_pool(name="psy", bufs=2, space="PSUM"))

        # ---- ALL loads on the sync HWDGE ring in dependency order:
        # consts first (sections; subtile deps let each consumer start as
        # soon as its own section lands), then x quarters (s0 first).
        # The scalar engine issues no DMAs so its full time goes to ACT.
        xsumt = cpool.tile([128, 2 * SPC], F32, tag="xsumt")
        nc.sync.dma_start(out=xsumt[:], in_=xsum_d[:, :])
        selb = cpool.tile([128, NSEL], F32, tag="selb")
        w16b = cpool.tile([128, N16], F16, tag="w16b")
        pairb = cpool.tile([128, 2 * CSE], F16, tag="pairb")
        for lo, hi in [(_WY32, _CMAT), (_CMAT, NSEL)]:
            nc.sync.dma_start(out=selb[:, lo:hi], in_=sel_d[:, lo:hi])
        for lo, hi in [(_TRIL, _WFOLD), (_WFOLD, N16)]:
            nc.sync.dma_start(out=w16b[:, lo:hi], in_=w16_d[:, lo:hi])
        X16 = [[x16p.tile([128, PIX], F16, name="xt") for _ in range(2)]
               for _ in range(SPC)]

        def load_x(s, hs):
            # 512KB half-tiles: large enough for DMA efficiency, small
            # enough for early consumer starts
            for h in hs:
                for kt in range(2):
                    r0 = s * C1 + kt * 128
                    nc.sync.dma_start(
                        out=X16[s][kt][:, h * 2048:(h + 1) * 2048],
                        in_=x_d[r0:r0 + 128, h * 2048:(h + 1) * 2048])

        load_x(0, [0, 1])
        nc.sync.dma_start(out=pairb[:], in_=pair_d[:, :])
        load_x(1, [0, 1])

        # ---- HAM warm-up: ~5us of dense dummy matmuls while the input
        # DMAs land (PE would be idle anyway). Forces the clock gate to
        # 8/8 before the real compute starts; without it the kernel is
        # bimodal (83us warm-start vs 99us cold-start).
        wrm = cpool.tile([128, 512], F16, tag="wrm")
        nc.vector.memset(wrm[:], 0.0)
        for _ in range(12):
            pw = psp.tile([128, 512], F32, tag="mm", name="pwarm")
            nc.tensor.matmul(pw[:], lhsT=wrm[:, :128], rhs=wrm[:],
                             start=True, stop=True)

        def c32(col, w):
            return selb[:, col:col + w]

        wyT32s = [c32(_WY32 + k * 256, 256) for k in range(2)]
        cmat = [c32(_CMAT + k * 256, 256) for k in range(2)]
        bcol = [c32(_BCOL + k, 1) for k in range(2)]
        shcol = [c32(_SHC + m, 1) for m in range(MT4)]
        tril = [w16b[:, _TRIL + k * 256: _TRIL + (k + 1) * 256]
                for k in range(2)]
        offd = [w16b[:, _OFFD + k * 256: _OFFD + (k + 1) * 256]
                for k in range(2)]
        iota128 = w16b[:, _IOTA:_IOTA + 128]
        wyT16 = [w16b[:, _WY16 + k * 256: _WY16 + (k + 1) * 256] for k in range(2)]
        wfold16 = [w16b[:, _WFOLD + k * 256: _WFOLD + (k + 1) * 256]
                   for k in range(2)]
        bcol16 = [w16b[:, _BC16 + k: _BC16 + k + 1] for k in range(2)]
        pS_stk = pairb[:, 0:CSE]
        pQn_stk = pairb[:, CSE:2 * CSE]

        WSEL4 = [None] * SPC     # [ct] -> [128,128] fp16 (4 copies of wsel)
        SBIAS = [None] * SPC     # [128,1] f32 stacked selected-bias
        XSEL = [None] * SPC      # stacked [128, 1024] fp16
        XSQ = [None] * SPC
        YSB = [[None] * 2 for _ in range(SPC)]
        ZO = [[None] * MT4 for _ in range(SPC)]

        ST4 = [[None, None] for _ in range(SPC)]

        def ph_sel_a():
            # Batched over both samples (N=2) to halve the count of
            # overhead-dominated fp32 LOW_HIGH matmuls.
            # pooled = W'@xbar + b' (exact f32; wyT32s folds the /4096)
            PB = []
            for mt in range(2):
                pp = psy.tile([128, SPC], F32, tag="mmy")
                for kt in range(2):
                    nc.tensor.matmul(
                        pp[:], lhsT=wyT32s[kt][:, mt * 128:(mt + 1) * 128],
                        rhs=xsumt[:, kt * SPC:(kt + 1) * SPC],
                        start=(kt == 0), stop=(kt == 1))
                pb = smallp.tile([128, SPC], F32, tag="pooled")
                nc.scalar.activation(pb[:], pp[:], AF.Identity,
                                     bias=bcol[mt], scale=1.0)
                PB.append(pb)
                if dbg:
                    for s in range(SPC):
                        nc.sync.dma_start(
                            out=dbg_pooled[s * C1 + mt * 128:
                                           s * C1 + (mt + 1) * 128, :],
                            in_=pb[:, s:s + 1])

            LC = []
            for qt in range(2):
                lc_ps = psy.tile([128, SPC], F32, tag="mmy")
                for ot in range(2):
                    nc.tensor.matmul(
                        lc_ps[:], lhsT=cmat[ot][:, qt * 128:(qt + 1) * 128],
                        rhs=PB[ot][:], start=(ot == 0), stop=(ot == 1))
                # copy to SBUF so the psy slot frees immediately (holding
                # it across both samples' rank chains blocks y units)
                lcol_sb = smallp.tile([128, SPC], F32, tag="lcol")
                nc.scalar.copy(lcol_sb[:], lc_ps[:])
                LC.append(lcol_sb)

            for s in range(SPC):
                # brow[c,b] = logits[b] for all c, via a broadcast-lhsT
                # matmul (lhsT = pooled column replicated across 128 cols):
                # brow = pooled_bcast^T @ cmat.  Same ~1e-7 fp32 path
                # rounding as lcol's matmul; offd masks the diagonal.
                brow = psp.tile([128, C1], F32, tag="mm", name="brow")
                for ot in range(2):
                    nc.tensor.matmul(
                        brow[:], lhsT=PB[ot][:, s:s + 1].broadcast_to([128, 128]),
                        rhs=cmat[ot], start=(ot == 0), stop=(ot == 1))
                for qt in range(2):
                    lcol = LC[qt][:, s:s + 1]
                    # rank[a] = #{b!=a: logits[b] > logits[a]}
                    #        + #{b<a: logits[b] == logits[a]} (jax tie-break)
                    # offd masks the diagonal: lrow/lcol come from different
                    # LOW_HIGH matmul decompositions, so brow[a,a] vs
                    # lcol[a] can differ by ~1e-7 and misfire is_gt.
                    g2 = gp.tile([128, C1], F32)
                    rank_t = smallp.tile([128, 1], F32, tag="rank_t")
                    nc.vector.scalar_tensor_tensor(
                        g2[:], brow[:], lcol, tril[qt],
                        op0=ALU.is_equal, op1=ALU.mult, accum_out=rank_t[:])
                    gsum = gp.tile([128, C1], F32)
                    rank_g = smallp.tile([128, 1], F32, tag="rank_g")
                    nc.vector.scalar_tensor_tensor(
                        gsum[:], brow[:], lcol, offd[qt],
                        op0=ALU.is_gt, op1=ALU.mult, accum_out=rank_g[:])
                    # S4_T[c, 32g+k] = ((k - rank_g[c]) == rank_t[c])
                    stq = smallp.tile([128, 128], F16, tag="st")
                    nc.vector.tensor_scalar(stq[:], iota128, rank_g[:],
                                            rank_t[:], op0=ALU.subtract,
                                            op1=ALU.is_equal)
                    ST4[s][qt] = stq
                    if dbg:
                        rank = smallp.tile([128, 1], F32, tag="rank")
                        nc.vector.tensor_tensor(rank[:], rank_g[:], rank_t[:],
                                                op=ALU.add)
                        r0 = s * C1 + qt * 128
                        nc.sync.dma_start(out=dbg_rank[r0:r0 + 128, :],
                                          in_=rank[:])

        def ph_sel_b(s):
            # selection weights (4 stacked copies):
            #   WSEL4[c, 32g+k] = sum_o W'[o,c] S4_T[o, 32g+k]
            st4 = ST4[s]
            wsel = []
            for ct in range(2):
                ws_ps = psp.tile([128, 128], F32, tag="mm")
                for ot in range(2):
                    nc.tensor.matmul(
                        ws_ps[:], lhsT=wfold16[ot][:, ct * 128:(ct + 1) * 128],
                        rhs=st4[ot][:], start=(ot == 0), stop=(ot == 1))
                wsq = smallp.tile([128, 128], F16, tag="wsel")
                nc.scalar.copy(wsq[:], ws_ps[:])
                wsel.append(wsq)
            WSEL4[s] = wsel
            sb_ps = psp.tile([128, 1], F32, tag="mm")
            for ot in range(2):
                nc.tensor.matmul(sb_ps[:], lhsT=st4[ot][:], rhs=bcol16[ot],
                                 start=(ot == 0), stop=(ot == 1))
            sbias = smallp.tile([128, 1], F32, tag="sbias")
            nc.scalar.copy(sbias[:], sb_ps[:])
            SBIAS[s] = sbias

        def ph_xsel(s):
            # x_sel (stacked [128,1024]): col-tiled 4x matmuls, partition
            # group g = selected channels for pixel block g.
            xsel = xselp.tile([128, 1024], F16, tag="xsel", name="xsel")
            for cc in range(2):
                psX = psp.tile([128, 512], F32, tag="mm", name="psX")
                for kt in range(2):
                    for g in range(4):
                        c0 = g * 1024 + cc * 512
                        nc.tensor.matmul(
                            psX[32 * g:32 * (g + 1), :],
                            lhsT=WSEL4[s][kt][:, 32 * g:32 * (g + 1)],
                            rhs=X16[s][kt][:, c0:c0 + 512],
                            start=(kt == 0), stop=(kt == 1),
                            tile_position=(0, 32 * g))
                dst = xsel[:, cc * 512:(cc + 1) * 512]
                if s == 0:  # keep the ramp-critical evac off the busy ACT
                    nc.vector.tensor_scalar(dst, psX[:], SBIAS[s][:], None,
                                            op0=ALU.add)
                else:
                    nc.scalar.activation(dst, psX[:], AF.Identity,
                                         bias=SBIAS[s][:], scale=1.0)
            XSEL[s] = xsel
            xsq = xselp.tile([128, 1024], F16, tag="xsq", name="xsq")
            for cc in range(2):  # split so cc=0 pair matmuls start early
                nc.vector.tensor_tensor(xsq[:, cc * 512:(cc + 1) * 512],
                                        xsel[:, cc * 512:(cc + 1) * 512],
                                        xsel[:, cc * 512:(cc + 1) * 512],
                                        op=ALU.mult)
            XSQ[s] = xsq
            if dbg:
                nc.sync.dma_start(out=dbg_xsel[s * 128:(s + 1) * 128, :],
                                  in_=xsel[:])

        def ph_y(s, mt, u, evac_dve):
            # y = W'x + b' over pixel chunk u (512 cols; fp16 mm, f32 psum).
            if u == 0:
                YSB[s][mt] = ysbp.tile([128, PIX], F16, tag="ysb", name="ysb")
            y_sb = YSB[s][mt]
            psY = psy.tile([128, 512], F32, tag="mmy", name="psY")
            for kt in range(2):
                nc.tensor.matmul(
                    psY[:], lhsT=wyT16[kt][:, mt * 128:(mt + 1) * 128],
                    rhs=X16[s][kt][:, u * 512:(u + 1) * 512],
                    start=(kt == 0), stop=(kt == 1))
            dst = y_sb[:, u * 512:(u + 1) * 512]
            if evac_dve:
                nc.vector.tensor_scalar(dst, psY[:], bcol[mt], None,
                                        op0=ALU.add)
            else:
                nc.scalar.activation(dst, psY[:], AF.Identity,
                                     bias=bcol[mt], scale=1.0)
            if u in (3, 7):
                r0 = s * COUT + mt * 128
                c0 = (u - 3) * 512
                nc.gpsimd.dma_start(out=out_d[r0:r0 + 128, c0:c0 + 2048],
                                    in_=y_sb[:, c0:c0 + 2048])

        def ph_z(s, m, cc, split_dma=False):
            # z = Square(sqrt(ss/2)(xi+xj)) + [-(ss/2)(xi^2+xj^2) + sh]
            # Row-tiled 4x: group g computes pixel block g; psS/psQ slots
            # hold 2 groups each ([128,1024] = 2 banks).
            p = min(128, CSE - m * 128)
            if cc == 0:
                ZO[s][m] = zop.tile([128, MT4, 1024], F16, tag="zo", name="zo")
            zo = ZO[s][m]
            # 4-way row-tiled S wave into two 2-bank slots (4 distinct
            # banks), then Q wave; FD=1024 evacuation ops.
            psS = [psp.tile([128, 1024], F32, tag="mm", name="psS")
                   for _ in range(2)]
            for g in range(4):
                nc.tensor.matmul(
                    psS[g // 2][:p, (g % 2) * 512:(g % 2 + 1) * 512],
                    lhsT=pS_stk[32 * g:32 * (g + 1), m * 128:m * 128 + p],
                    rhs=XSEL[s][32 * g:32 * (g + 1), cc * 512:(cc + 1) * 512],
                    start=True, stop=True, tile_position=(32 * g, 0))
            psQ = [psp.tile([128, 1024], F32, tag="mm", name="psQ")
                   for _ in range(2)]
            for g in range(4):
                nc.tensor.matmul(
                    psQ[g // 2][:p, (g % 2) * 512:(g % 2 + 1) * 512],
                    lhsT=pQn_stk[32 * g:32 * (g + 1), m * 128:m * 128 + p],
                    rhs=XSQ[s][32 * g:32 * (g + 1), cc * 512:(cc + 1) * 512],
                    start=True, stop=True, tile_position=(32 * g, 0))
            for half in range(2):
                sq = sqp.tile([128, 1024], F16, tag="sq", name="sq")
                nc.scalar.activation(sq[:p, :], psS[half][:p, :], AF.Square)
                # z = (psQ + sh) + sq into pixel blocks (2h, 2h+1), col cc
                zv = zo[:p, 2 * half:2 * half + 2, cc * 512:(cc + 1) * 512]
                nc.vector.scalar_tensor_tensor(
                    zv, psQ[half][:p, :].rearrange("q (b n) -> q b n", b=2),
                    shcol[m][:p], sq[:p, :].rearrange("q (b n) -> q b n", b=2),
                    op0=ALU.add, op1=ALU.add)
            r0 = s * COUT + C1 + m * 128
            if split_dma:
                # drain-tail units: ship each column-half as soon as ready
                nc.gpsimd.dma_start(
                    out=out_d[r0:r0 + p, :].rearrange(
                        "q (g c n) -> q g c n", g=4, c=2)[:, :, cc],
                    in_=zo[:p, :, cc * 512:(cc + 1) * 512])
            elif cc == 1:
                nc.gpsimd.dma_start(
                    out=out_d[r0:r0 + p, :],
                    in_=zo[:p].rearrange("q g n -> q (g n)"))

        # ---- emission order (per-engine FIFO order) ----
        # sel chain batched+early; then z and y units interleaved to keep
        # the PE densely busy (HAM warmth).
        yunits = [(s, mt, u) for s in range(SPC) for mt in range(2)
                  for u in range(8)]
        # s0 cc=0 units first (they only need the cc=0 half of xsel(0)),
        # cc=1 and s1 units follow; each (s,m) keeps cc0 before cc1.
        zunits = [(0, 0, 0), (0, 1, 0), (0, 2, 0), (0, 3, 0),
                  (0, 0, 1), (0, 1, 1), (1, 0, 0), (0, 2, 1),
                  (1, 1, 0), (0, 3, 1), (1, 2, 0), (1, 0, 1),
                  (1, 3, 0), (1, 1, 1), (1, 2, 1), (1, 3, 1)]
        yi = 0

        def emit_y(n):
            nonlocal yi
            for _ in range(n):
                if yi < len(yunits):
                    s, mt, u = yunits[yi]
                    # early y units evac on DVE (ACT is busy with the sel
                    # chain during the ramp); later ones mostly on ACT
                    dve = (yi % 2 == 0) if yi < 8 else (yi % 4 == 2)
                    ph_y(s, mt, u, dve)
                    yi += 1

        ph_sel_a()
        ph_sel_b(0)
        ph_xsel(0)
        ph_z(*zunits[0])
        ph_sel_b(1)
        # xsel(1) is deferred until just before the first s1 z unit: its
        # matmuls block on the late-arriving x(s1) DMA, and the strict PE
        # FIFO would stall every s0 unit emitted behind them.
        for i, (s, m, cc) in enumerate(zunits[1:]):
            if s == 1 and XSEL[1] is None:
                ph_xsel(1)
            emit_y(2)
            ph_z(s, m, cc, split_dma=(s, m) in ((1, 2), (1, 3)))
        emit_y(99)

_CACHE = {}


def _get_nc(dbg: bool = False):
    key = f"nc{int(dbg)}"
    if key not in _CACHE:
        nc = bacc.Bacc("TRN2", target_bir_lowering=False, debug=False,
                       num_devices=NCORES)
        _build(nc, dbg=dbg)
        nc.compile()
        _CACHE[key] = nc
    return _CACHE[key]


def _host_params(w_fc, b_fc, g_x, b_x, m_x, v_x, eca_w, g_s, b_s, m_s, v_s):
    sx = (g_x / np.sqrt(v_x + EPS)).astype(np.float32)            # [256]
    Wp = (sx[:, None] * w_fc).astype(np.float32)                  # [o, c]
    bp = (sx * b_fc + b_x - m_x * sx).astype(np.float32)          # [256]

    cmat = np.zeros((C1, C1), np.float32)                         # [o, q]
    for k in range(5):
        d = k - 2                                                 # o - q
        for q in range(C1):
            o = q + d
            if 0 <= o < C1:
                cmat[o, q] = eca_w[k]

    tril = (np.arange(C1)[None, :] < np.arange(C1)[:, None]).astype(np.float32)

    hi, hj = np.triu_indices(CS, k=1)
    ss = (g_s / np.sqrt(v_s + EPS)).astype(np.float32)
    sh = (b_s - m_s * ss).astype(np.float32)
    # squares-trick pair matrices (stacked 4x across partition groups):
    #   psS = pS.T @ xsel with pS[i,pq] = sqrt(ss/2) * [i in (hi,hj)]
    #   psQ = pQn.T @ xsel^2 with pQn[i,pq] = -(ss/2)*[i in (hi,hj)]
    #   z = Square(psS) + (psQ + sh)
    ar = np.arange(CSE)
    inc = np.zeros((CS, CSE), np.float32)
    inc[hi, ar] = 1.0
    inc[hj, ar] += 1.0
    pS = (inc * np.sqrt(ss / 2.0)[None, :]).astype(np.float16)
    pQn = (inc * (-ss / 2.0)[None, :]).astype(np.float16)

    iota128 = np.tile(np.tile(np.arange(CS, dtype=np.float32), 4), (128, 1))

    return {
        "wyT16": Wp.T.astype(np.float16).copy(),
        "wyT32s": (Wp.T / PIX).astype(np.float32).copy(),
        "wfold16": Wp.astype(np.float16).copy(),
        "bcol": bp.reshape(C1, 1).copy(),
        "bcol16": bp.astype(np.float16).reshape(C1, 1).copy(),
        "cmat": cmat,
        "tril": tril,
        "offd": (1.0 - np.eye(C1, dtype=np.float32)),
        "iota128": iota128,
        "pS_stk": np.tile(pS, (4, 1)).copy(),                     # [128, 496]
        "pQn_stk": np.tile(pQn, (4, 1)).copy(),
        "shcol": sh.reshape(CSE, 1).copy(),
    }


def _semantic_params(inputs):
    return _host_params(
        np.asarray(inputs["w_fc"], np.float32),
        np.asarray(inputs["b_fc"], np.float32),
        np.asarray(inputs["bn_x_gamma"], np.float32),
        np.asarray(inputs["bn_x_beta"], np.float32),
        np.asarray(inputs["bn_x_mean"], np.float32),
        np.asarray(inputs["bn_x_var"], np.float32),
        np.asarray(inputs["eca_w"], np.float32),
        np.asarray(inputs["bn_s_gamma"], np.float32),
        np.asarray(inputs["bn_s_beta"], np.float32),
        np.asarray(inputs["bn_s_mean"], np.float32),
        np.asarray(inputs["bn_s_var"], np.float32),
    )


def _pack_blobs(P):
    """Pack semantic params into the const blobs matching _build's layout."""
    selb = np.zeros((128, NSEL), np.float32)
    for k in range(2):
        selb[:, _WY32 + k * 256: _WY32 + (k + 1) * 256] = \
            P["wyT32s"][k * 128:(k + 1) * 128]
        selb[:, _CMAT + k * 256: _CMAT + (k + 1) * 256] = \
            P["cmat"][k * 128:(k + 1) * 128]
        selb[:, _BCOL + k] = P["bcol"][k * 128:(k + 1) * 128, 0]
    for m in range(MT4):
        p = min(128, CSE - m * 128)
        selb[:p, _SHC + m] = P["shcol"][m * 128: m * 128 + p, 0]

    w16 = np.zeros((128, N16), np.float16)
    for k in range(2):
        w16[:, _TRIL + k * 256: _TRIL + (k + 1) * 256] = \
            P["tril"][k * 128:(k + 1) * 128]
        w16[:, _OFFD + k * 256: _OFFD + (k + 1) * 256] = \
            P["offd"][k * 128:(k + 1) * 128]
        w16[:, _WY16 + k * 256: _WY16 + (k + 1) * 256] = \
            P["wyT16"][k * 128:(k + 1) * 128]
        w16[:, _WFOLD + k * 256: _WFOLD + (k + 1) * 256] = \
            P["wfold16"][k * 128:(k + 1) * 128]
        w16[:, _BC16 + k] = P["bcol16"][k * 128:(k + 1) * 128, 0]
    w16[:, _IOTA:_IOTA + 128] = P["iota128"]

    pairb = np.concatenate([P["pS_stk"], P["pQn_stk"]], axis=1)
    return {"selblob": selb, "w16blob": np.ascontiguousarray(w16),
            "pairblob": np.ascontiguousarray(pairb.astype(np.float16))}


def _in_maps(inputs):
    x = np.ascontiguousarray(np.asarray(inputs["x"], np.float32))
    blobs = _pack_blobs(_semantic_params(inputs))
    maps = []
    for c in range(NCORES):
        shard = x[c * SPC:(c + 1) * SPC].reshape(SPC * C1, PIX)
        # exact f32 per-channel pixel sums (feeds the pooled/top-k path);
        # fp16 x feeds the matmuls
        xsum = shard.sum(axis=1, dtype=np.float32)          # [512]
        xsumv = np.zeros((128, 2 * SPC), np.float32)        # col = kt*SPC+s
        for s in range(SPC):
            for kt in range(2):
                xsumv[:, kt * SPC + s] = xsum[s * C1 + kt * 128:
                                              s * C1 + (kt + 1) * 128]
        maps.append({"x16v": shard.astype(np.float16),
                     "xsumv": xsumv, **blobs})
    return maps


def _ensure_ntff_hook():
    """The agent image lacks antenv.axon_hooks; synthesize it so
    run_bass_kernel_spmd(trace=True) can reach the NTFF profiler in
    libaxon_pjrt.so. Safe no-op if anything is missing."""
    try:
        import antenv.axon_hooks  # noqa: F401
        return
    except ImportError:
        pass
    try:
        import types
        import antenv
        from trn_agent_boot.trn_boot import _ntff_profile_via_ctypes
        hook = _ntff_profile_via_ctypes("/opt/axon/libaxon_pjrt.so")
        mod = types.ModuleType("antenv.axon_hooks")
        mod._hook = hook
        mod.get_axon_ntff_profile_hook = lambda: mod._hook
        mod.set_axon_ntff_profile_hook = lambda h: setattr(mod, "_hook", h)
        sys.modules["antenv.axon_hooks"] = mod
        antenv.axon_hooks = mod
    except Exception as e:  # pragma: no cover
        print(f"ntff hook shim failed: {e}", file=sys.stderr)


def run(inputs, trace=False, dbg=False):
    if trace:
        _ensure_ntff_hook()
    nc = _get_nc(dbg=dbg)
    maps = _in_maps(inputs)
    res = run_bass_kernel_spmd(nc, maps, core_ids=list(range(NCORES)),
                               trace=trace)
    outs = [np.asarray(res.results[c]["out16"]).astype(np.float32)
            .reshape(SPC, COUT, H, W) for c in range(NCORES)]
    return np.concatenate(outs, axis=0), res


def kernel(**inputs) -> np.ndarray:
    out, _ = run(inputs, trace=False)
    return out
